# revision 19
# baseline (speedup 1.0000x reference)
"""Trainium2 Bass kernel for nn_MultiHeadAttention_37477884625313.

Multi-head attention (B=4, C=512, T=1024, H=8, d=64) with the reference's
relative-position terms:
  - score-side rel term lands at 21 corner entries per head
  - ctx-side rel term is the +/-4 diagonal band of attn with emb_v

Sharding: 8 cores = 4 batches x 2 query-halves.  k/v time axes are
host-rotated by -i0 so the SPMD program is identical across cores.

v2: scores are computed j-major (keys on partitions) so no transposes are
needed before the ctx matmul; the softmax denominator Z comes from a ones
column appended to the v stationary; exp is the only scalar-engine work;
the rel_v band is gathered via a DRAM diagonal read on the gpsimd queue.
"""

import sys

sys.path.insert(0, "/opt/trn_rl_repo")

import numpy as np
import ml_dtypes

import concourse.bass as bass
import concourse.mybir as mybir
import concourse.tile as tile
from concourse import bacc
from concourse.ap import AP
from concourse.bass_utils import run_bass_kernel_spmd

BF16 = ml_dtypes.bfloat16
P = 128
C = 512
T = 1024
H = 8
D = 64
TQ = 512          # queries per core
IB = 4            # query i-blocks per core
NCT = 4           # channel tiles (512 / 128)
WJ = 136          # band j-window rows per i-block (4 + 128 + 4)
EXP = mybir.ActivationFunctionType.Exp

_CACHE = {}


def build_nc():
    nc = bacc.Bacc("TRN2", target_bir_lowering=False)
    f32, bf = mybir.dt.float32, mybir.dt.bfloat16
    add_op = mybir.AluOpType.add
    mul_op = mybir.AluOpType.mult

    xq = nc.declare_dram_parameter("xq", [P, NCT, TQ], bf, isOutput=False)
    xk = nc.declare_dram_parameter("xk", [P, NCT, T], bf, isOutput=False)
    xv = nc.declare_dram_parameter("xv", [P, NCT, T], bf, isOutput=False)
    wqT = nc.declare_dram_parameter("wqT", [P, NCT, C], bf, isOutput=False)
    wkT = nc.declare_dram_parameter("wkT", [P, NCT, C], bf, isOutput=False)
    wvT = nc.declare_dram_parameter("wvT", [P, NCT, C], bf, isOutput=False)
    woT = nc.declare_dram_parameter("woT", [P, NCT, C], bf, isOutput=False)
    bq8 = nc.declare_dram_parameter("bq8", [P, NCT], f32, isOutput=False)
    evh = nc.declare_dram_parameter("evh", [18, H, 65], bf, isOutput=False)
    fp16 = mybir.dt.float16
    oneh2 = nc.declare_dram_parameter("oneh2", [2, P], fp16, isOutput=False)
    cor1d = nc.declare_dram_parameter("cor1d", [32, H, 5], bf, isOutput=False)
    cor2d = nc.declare_dram_parameter("cor2d", [32, H, 3], bf, isOutput=False)
    maskd = nc.declare_dram_parameter("maskd", [18, TQ], bf, isOutput=False)
    Out = nc.declare_dram_parameter("Out", [NCT, P, TQ], f32, isOutput=True)

    # band staging: per pair a: 1028 j rows (j+4, incl 4-row wrap halo) x 2 heads x 512 i
    stg = nc.dram_tensor("stg", [NCT, 1028, 2, TQ], bf)

    with tile.TileContext(nc) as tc:
        with (
            tc.tile_pool(name="persist", bufs=1) as pp,
            tc.tile_pool(name="epool", bufs=2) as ep,
            tc.tile_pool(name="work", bufs=2) as wp,
            tc.tile_pool(name="psP", bufs=2, space="PSUM") as psP,
            tc.tile_pool(name="psS", bufs=2, space="PSUM") as psS,
            tc.tile_pool(name="psC", bufs=2, space="PSUM") as psC,
        ):
            # ---- persistent tiles --------------------------------------
            xq_sb = pp.tile([P, NCT, TQ], bf, name="xq_sb")
            xk_sb = pp.tile([P, NCT, T], bf, name="xk_sb")
            xv_sb = pp.tile([P, NCT, T], bf, name="xv_sb")
            wq_sb = pp.tile([P, NCT, C], bf, name="wq_sb")
            wk_sb = pp.tile([P, NCT, C], bf, name="wk_sb")
            wv_sb = pp.tile([P, NCT, C], bf, name="wv_sb")
            wo_sb = pp.tile([P, NCT, C], bf, name="wo_sb")
            bq_sb = pp.tile([P, NCT], f32, name="bq_sb")
            ev_sb = pp.tile([18, H, 65], bf, name="ev_sb")
            o2_sb = pp.tile([P, P], fp16, name="o2_sb")
            cor1 = pp.tile([P, H, 5], bf, name="cor1")
            cor2 = pp.tile([32, H, 3], bf, name="cor2")
            mk_sb = pp.tile([18, TQ], bf, name="mk_sb")

            q_sb = pp.tile([P, NCT, TQ], bf, name="q_sb")
            k_sb = pp.tile([P, NCT, T], bf, name="k_sb")
            vT65 = pp.tile([P, 8, H, 65], bf, name="vT65")  # [j, tb, h, ch|1]
            ctxn = pp.tile([P, NCT, TQ], bf, name="ctxn")

            # ---- input loads, in first-use order -----------------------
            nc.sync.dma_start(xq_sb[:], xq[:])
            nc.sync.dma_start(wq_sb[:], wqT[:])
            nc.sync.dma_start(bq_sb[:], bq8[:])
            nc.sync.dma_start(xk_sb[:], xk[:])
            nc.sync.dma_start(wk_sb[:], wkT[:])
            nc.sync.dma_start(xv_sb[:], xv[:])
            nc.sync.dma_start(wv_sb[:], wvT[:])
            nc.sync.dma_start(ev_sb[:], evh[:])
            nc.sync.dma_start(o2_sb[64:66, :], oneh2[:])
            nc.sync.dma_start(cor1[96:128, :, :], cor1d[:])
            nc.sync.dma_start(cor2[:], cor2d[:])
            nc.sync.dma_start(mk_sb[:], maskd[:])
            nc.sync.dma_start(wo_sb[:], woT[:])

            # ones column of the v stationary (Z accumulator).  v and ev are
            # host-scaled by 1/64 and the ones column holds 1/64 so that
            # 1/Z' = 64/Z stays in fp16 normal range for the broadcast matmul;
            # the factors cancel exactly in ctxn = (ctx/64) * (64/Z).
            nc.vector.memset(vT65[:, :, :, 64:65], 1.0 / 64.0)

            # ---- helpers ----------------------------------------------
            def projqk(ct):
                pq = psP.tile([P, TQ], f32, name="pj")
                for ci in range(NCT):
                    nc.tensor.matmul(pq, wq_sb[:, ci, ct * P:(ct + 1) * P],
                                     xq_sb[:, ci, :], start=(ci == 0), stop=(ci == 3))
                nc.vector.tensor_scalar_add(q_sb[:, ct, :], pq, bq_sb[:, ct:ct + 1])
                for ns in range(2):
                    pk = psP.tile([P, TQ], f32, name="pj")
                    for ci in range(NCT):
                        nc.tensor.matmul(pk, wk_sb[:, ci, ct * P:(ct + 1) * P],
                                         xk_sb[:, ci, ns * TQ:(ns + 1) * TQ],
                                         start=(ci == 0), stop=(ci == 3))
                    nc.vector.tensor_copy(k_sb[:, ct, ns * TQ:(ns + 1) * TQ], pk)

            def projv(tb):
                pv = psP.tile([P, TQ], f32, name="pj")
                for ci in range(NCT):
                    nc.tensor.matmul(pv, xv_sb[:, ci, tb * P:(tb + 1) * P],
                                     wv_sb[:, ci, :], start=(ci == 0), stop=(ci == 3))
                # pv: [128 t, 512 ch] -> vT65[:, tb, h, 0:64]
                nc.vector.tensor_copy(
                    vT65[:, tb, :, 0:64],
                    pv[:].rearrange("p (h c) -> p h c", h=H))

            e_tiles = {}

            def score_unit(a, jb):
                ps = psS.tile([P, 2 * TQ], f32, name="sc")
                for hh in range(2):
                    nc.tensor.matmul(ps[:, hh * TQ:(hh + 1) * TQ],
                                     k_sb[hh * D:(hh + 1) * D, a, jb * P:(jb + 1) * P],
                                     q_sb[hh * D:(hh + 1) * D, a, :],
                                     start=True, stop=True)
                if jb == 7:
                    for hh in range(2):
                        nc.vector.tensor_tensor(
                            out=ps[96:128, hh * TQ:hh * TQ + 5],
                            in0=ps[96:128, hh * TQ:hh * TQ + 5],
                            in1=cor1[96:128, 2 * a + hh, :], op=add_op)
                if jb == 4:
                    for hh in range(2):
                        nc.vector.tensor_tensor(
                            out=ps[0:32, hh * TQ + 509:hh * TQ + 512],
                            in0=ps[0:32, hh * TQ + 509:hh * TQ + 512],
                            in1=cor2[0:32, 2 * a + hh, :], op=add_op)
                e_a = e_tiles[a]
                nc.scalar.activation(
                    e_a[:, :, jb, :],
                    ps[:].rearrange("p (u q) -> p u q", u=2), EXP)

            def stage_band(a):
                e_a = e_tiles[a]
                abT = wp.tile([18, IB * P], bf, name="abT")
                for hh in range(2):
                    # rows 4..1027 <- e[j, i]; row = j + 4
                    nc.gpsimd.dma_start(
                        stg[a, 4:1028, hh, :].rearrange("(jb p) i -> p jb i", p=P),
                        e_a[:, hh, :, :])
                    # wrap halo rows 0..3 <- j 1020..1023
                    nc.gpsimd.dma_start(stg[a, 0:4, hh, :],
                                        e_a[124:128, hh, 7, :])
                # diagonal readback: abT[9*hh+m, i] = stg[a, i+m, hh, i]
                for hh in range(2):
                    diag = AP(tensor=stg[:].tensor,
                              offset=a * (1028 * 2 * TQ) + hh * TQ,
                              ap=[[2 * TQ, 9], [2 * TQ + 1, TQ]])
                    nc.gpsimd.dma_start(abT[9 * hh:9 * hh + 9, :], diag)
                nc.vector.tensor_tensor(out=abT[:], in0=abT[:], in1=mk_sb[:],
                                        op=mul_op)
                return abT

            def ctx_head(a, hh):
                e_a = e_tiles[a]
                h = 2 * a + hh
                pc = psC.tile([P, TQ], f32, name="pc")
                for jb in range(8):
                    nc.tensor.matmul(pc[0:65, :], vT65[:, jb, h, :],
                                     e_a[:, hh, jb, :],
                                     start=(jb == 0), stop=False,
                                     skip_group_check=True)
                return pc

            def band_head(a, hh, abT, pc):
                nc.tensor.matmul(pc[0:65, :], ev_sb[:, 2 * a + hh, :], abT[:],
                                 start=False, stop=True, skip_group_check=True)

            def finish_pair(a, pc0, pc1):
                z2 = wp.tile([P, TQ], f32, name="z2")
                zt = wp.tile([P, TQ], f32, name="zt")
                r2 = wp.tile([P, TQ], f32, name="r2")
                r16 = wp.tile([P, TQ], fp16, name="r16")
                # assemble both Z rows at partitions 64..65, one reciprocal
                nc.scalar.copy(z2[64:65, :], pc0[64:65, :])
                nc.scalar.copy(zt[64:65, :], pc1[64:65, :])
                nc.sync.dma_start(z2[65:66, :], zt[64:65, :])
                nc.vector.reciprocal(r2[64:66, :], z2[64:66, :])
                nc.vector.tensor_copy(r16[64:66, :], r2[64:66, :])
                pz = psP.tile([P, TQ], f32, name="pj")
                nc.tensor.matmul(pz, o2_sb[64:66, :], r16[64:66, :],
                                 start=True, stop=True)
                rzb = wp.tile([P, TQ], f32, name="rzb")
                nc.vector.tensor_copy(rzb[:], pz[:])
                # head 0: aligned multiply into ctxn rows 0..63
                nc.vector.tensor_tensor(out=ctxn[0:D, a, :], in0=pc0[0:D, :],
                                        in1=rzb[0:D, :], op=mul_op)
                # head 1: multiply at partitions 0..63, then DMA-shift to 64..127
                ctmp = wp.tile([D, TQ], bf, name="ctmp")
                nc.vector.tensor_tensor(out=ctmp[:], in0=pc1[0:D, :],
                                        in1=rzb[D:2 * D, :], op=mul_op)
                nc.sync.dma_start(ctxn[D:2 * D, a, :], ctmp[:])

            def oproj(co):
                po = psP.tile([P, TQ], f32, name="pj")
                for ci in range(NCT):
                    nc.tensor.matmul(po, wo_sb[:, ci, co * P:(co + 1) * P],
                                     ctxn[:, ci, :], start=(ci == 0), stop=(ci == 3))
                o_sb = wp.tile([P, TQ], f32, name="o_sb")
                nc.vector.tensor_copy(o_sb[:], po)
                nc.sync.dma_start(Out[co], o_sb[:])

            # ---- emission: software-pipelined --------------------------
            for a in range(NCT):
                e_tiles[a] = ep.tile([P, 2, 8, TQ], bf, name="e_a")

            projqk(0)
            score_unit(0, 0); score_unit(0, 1)
            projqk(1)
            score_unit(0, 2); score_unit(0, 3)
            projv(0); projv(1); projv(2); projv(3)
            score_unit(0, 4); score_unit(0, 5)
            projv(4); projv(5); projv(6); projv(7)
            score_unit(0, 6); score_unit(0, 7)

            projqk(2)
            for jb in range(4):
                score_unit(1, jb)
            abT0 = stage_band(0)
            pc00 = ctx_head(0, 0)
            score_unit(1, 4); score_unit(1, 5)
            pc01 = ctx_head(0, 1)
            band_head(0, 0, abT0, pc00); band_head(0, 1, abT0, pc01)
            score_unit(1, 6); score_unit(1, 7)
            finish_pair(0, pc00, pc01)

            projqk(3)
            for jb in range(4):
                score_unit(2, jb)
            abT1 = stage_band(1)
            pc10 = ctx_head(1, 0)
            score_unit(2, 4); score_unit(2, 5)
            pc11 = ctx_head(1, 1)
            band_head(1, 0, abT1, pc10); band_head(1, 1, abT1, pc11)
            score_unit(2, 6); score_unit(2, 7)
            finish_pair(1, pc10, pc11)

            for jb in range(4):
                score_unit(3, jb)
            abT2 = stage_band(2)
            pc20 = ctx_head(2, 0)
            score_unit(3, 4); score_unit(3, 5)
            pc21 = ctx_head(2, 1)
            band_head(2, 0, abT2, pc20); band_head(2, 1, abT2, pc21)
            score_unit(3, 6); score_unit(3, 7)
            finish_pair(2, pc20, pc21)

            abT3 = stage_band(3)
            pc30 = ctx_head(3, 0)
            pc31 = ctx_head(3, 1)
            band_head(3, 0, abT3, pc30); band_head(3, 1, abT3, pc31)
            finish_pair(3, pc30, pc31)

            for co in range(NCT):
                oproj(co)

    nc.compile()
    return nc


def _prep(inputs):
    x_q = np.asarray(inputs["x_q"], np.float32)
    x_k = np.asarray(inputs["x_k"], np.float32)
    x_v = np.asarray(inputs["x_v"], np.float32)
    Wq = np.asarray(inputs["Wq"], np.float32)
    Wk = np.asarray(inputs["Wk"], np.float32)
    Wv = np.asarray(inputs["Wv"], np.float32)
    Wo = np.asarray(inputs["Wo"], np.float32)
    bq = np.asarray(inputs["bq"], np.float32)
    ek = np.asarray(inputs["emb_rel_k"], np.float32)
    ev = np.asarray(inputs["emb_rel_v"], np.float32)

    def ctile(a):  # (C, X) -> (P, NCT, X) partition-first
        return np.ascontiguousarray(a.reshape(NCT, P, -1).transpose(1, 0, 2))

    wqT = ctile(Wq.T * 0.125).astype(BF16)
    wkT = ctile(Wk.T).astype(BF16)
    wvT = ctile(Wv.T * (1.0 / 64.0)).astype(BF16)
    woT = ctile(Wo.T).astype(BF16)
    bq8 = np.ascontiguousarray((bq * 0.125).reshape(NCT, P).T).astype(np.float32)

    # band stationary per head: rows 9*(h%2)+m carry ev[h, m, :], col 64 = 0
    evh = np.zeros((18, H, 65), np.float32)
    for h in range(H):
        hh = h % 2
        for m in range(9):
            evh[9 * hh + m, h, 0:D] = ev[h, m] * (1.0 / 64.0)
    evh = evh.astype(BF16)

    # broadcast stationary (lives at sbuf partitions 64..65)
    oneh2 = np.zeros((2, P), np.float16)
    oneh2[0, 0:D] = 1.0
    oneh2[1, D:2 * D] = 1.0

    in_maps = []
    for core in range(8):
        b, half = core // 2, core % 2
        i0 = half * TQ
        # corner score terms, computed on host from a few q columns
        cor1 = np.zeros((P, H, 5), np.float32)
        cor2 = np.zeros((P, H, 3), np.float32)
        if half == 0:
            qc = (Wq @ x_q[b][:, 0:5]) * 0.125 + (bq[:, None] * 0.125)
            for h in range(H):
                for p in range(5):
                    for c in range(p, 5):
                        # j = 1019 + c, i = p, emb row m = c - p
                        cor1[p, h, c] = qc[h * D:(h + 1) * D, p] @ ek[h, c - p]
        else:
            qc = (Wq @ x_q[b][:, 1019:1023]) * 0.125 + (bq[:, None] * 0.125)
            for h in range(H):
                for p in (125, 126, 127):
                    for c in range(3):
                        m = 133 + c - p
                        if 6 <= m <= 8:
                            # i_glob = 896 + p, uses q_{i-1} = col 895 + p
                            cor2[p, h, c] = qc[h * D:(h + 1) * D, (895 + p) - 1019] @ ek[h, m]
        # j-major transposed corner tables, zero-padded to 32 aligned rows
        cor1T = np.zeros((32, H, 5), np.float32)
        cor1T[27:32] = cor1[0:5].transpose(2, 1, 0)     # rows 123..127 <- [cj, h, i]
        cor2T = np.zeros((32, H, 3), np.float32)
        cor2T[0:3] = cor2[125:128].transpose(2, 1, 0)   # rows 0..2    <- [cj, h, i]

        # band validity mask: row 9*hh+m, col i_abs
        maskT = np.zeros((18, TQ), np.float32)
        ivec = np.arange(TQ)
        for m in range(9):
            ok = ((i0 + ivec + m - 4 >= 0) & (i0 + ivec + m - 4 < T))
            maskT[m, :] = ok
            maskT[9 + m, :] = ok

        in_maps.append({
            "xq": ctile(x_q[b][:, i0:i0 + TQ]).astype(BF16),
            "xk": ctile(np.roll(x_k[b], -i0, axis=1)).astype(BF16),
            "xv": ctile(np.roll(x_v[b], -i0, axis=1)).astype(BF16),
            "wqT": wqT, "wkT": wkT, "wvT": wvT, "woT": woT,
            "bq8": bq8, "evh": evh, "oneh2": oneh2,
            "cor1d": cor1T.astype(BF16), "cor2d": cor2T.astype(BF16),
            "maskd": maskT.astype(BF16),
        })
    return in_maps


def kernel(**inputs):
    if "nc" not in _CACHE:
        _CACHE["nc"] = build_nc()
    nc = _CACHE["nc"]
    in_maps = _prep(inputs)
    res = run_bass_kernel_spmd(nc, in_maps, list(range(8)))
    bo = np.asarray(inputs["bo"], np.float32)
    bv = np.asarray(inputs["bv"], np.float32)
    Wo = np.asarray(inputs["Wo"], np.float32)
    bo_eff = bo + Wo @ bv
    out = np.zeros((4, C, T), np.float32)
    for core in range(8):
        b, half = core // 2, core % 2
        o = np.asarray(res.results[core]["Out"]).reshape(C, TQ)
        out[b][:, half * TQ:(half + 1) * TQ] = o
    out += bo_eff[None, :, None]
    return out


# revision 27
# speedup vs baseline: 1.1046x; 1.1046x over previous
"""Trainium2 Bass kernel for nn_MultiHeadAttention_37477884625313.

Multi-head attention (B=4, C=512, T=1024, H=8, d=64) with the reference's
relative-position terms:
  - score-side rel term lands at 21 corner entries per head
  - ctx-side rel term is the +/-4 diagonal band of attn with emb_v

Sharding: 8 cores = 4 batches x 2 query-halves.  k/v time axes are
host-rotated by -i0 so the SPMD program is identical across cores.

v2: scores are computed j-major (keys on partitions) so no transposes are
needed before the ctx matmul; the softmax denominator Z comes from a ones
column appended to the v stationary; exp is the only scalar-engine work;
the rel_v band is gathered via a DRAM diagonal read on the gpsimd queue.
"""

import sys

sys.path.insert(0, "/opt/trn_rl_repo")

import numpy as np
import ml_dtypes

import concourse.bass as bass
import concourse.mybir as mybir
import concourse.tile as tile
from concourse import bacc
from concourse.ap import AP
from concourse.bass_utils import run_bass_kernel_spmd

BF16 = ml_dtypes.bfloat16
P = 128
C = 512
T = 1024
H = 8
D = 64
TQ = 512          # queries per core
IB = 4            # query i-blocks per core
NCT = 4           # channel tiles (512 / 128)
WJ = 136          # band j-window rows per i-block (4 + 128 + 4)
EXP = mybir.ActivationFunctionType.Exp

_CACHE = {}


def build_nc():
    nc = bacc.Bacc("TRN2", target_bir_lowering=False)
    f32, bf = mybir.dt.float32, mybir.dt.bfloat16
    add_op = mybir.AluOpType.add
    mul_op = mybir.AluOpType.mult

    xq = nc.declare_dram_parameter("xq", [P, NCT, TQ], bf, isOutput=False)
    xk = nc.declare_dram_parameter("xk", [P, NCT, T], bf, isOutput=False)
    xv = nc.declare_dram_parameter("xv", [P, NCT, T], bf, isOutput=False)
    wqT = nc.declare_dram_parameter("wqT", [P, NCT, C], bf, isOutput=False)
    wkT = nc.declare_dram_parameter("wkT", [P, NCT, C], bf, isOutput=False)
    wvT = nc.declare_dram_parameter("wvT", [P, NCT, C], bf, isOutput=False)
    woT = nc.declare_dram_parameter("woT", [P, NCT, C], bf, isOutput=False)
    bq8 = nc.declare_dram_parameter("bq8", [P, NCT], f32, isOutput=False)
    evh = nc.declare_dram_parameter("evh", [9, H, 65], bf, isOutput=False)
    fp16 = mybir.dt.float16
    oneh2 = nc.declare_dram_parameter("oneh2", [2, P], fp16, isOutput=False)
    cor1d = nc.declare_dram_parameter("cor1d", [32, H, 5], bf, isOutput=False)
    cor2d = nc.declare_dram_parameter("cor2d", [32, H, 3], bf, isOutput=False)
    maskd = nc.declare_dram_parameter("maskd", [9, TQ], bf, isOutput=False)
    Out = nc.declare_dram_parameter("Out", [NCT, P, TQ], f32, isOutput=True)

    # band staging: per pair a: 1028 j rows (j+4, incl 4-row wrap halo) x 2 heads x 512 i
    stg = nc.dram_tensor("stg", [NCT, 1028, 2, TQ], bf)

    with tile.TileContext(nc) as tc:
        with (
            tc.tile_pool(name="persist", bufs=1) as pp,
            tc.tile_pool(name="epool", bufs=2) as ep,
            tc.tile_pool(name="work", bufs=2) as wp,
            tc.tile_pool(name="psP", bufs=2, space="PSUM") as psP,
            tc.tile_pool(name="psS", bufs=2, space="PSUM") as psS,
            tc.tile_pool(name="psC", bufs=2, space="PSUM") as psC,
        ):
            # ---- persistent tiles --------------------------------------
            xq_sb = pp.tile([P, NCT, TQ], bf, name="xq_sb")
            xk_sb = pp.tile([P, NCT, T], bf, name="xk_sb")
            xv_sb = pp.tile([P, NCT, T], bf, name="xv_sb")
            wq_sb = pp.tile([P, NCT, C], bf, name="wq_sb")
            wk_sb = pp.tile([P, NCT, C], bf, name="wk_sb")
            wv_sb = pp.tile([P, NCT, C], bf, name="wv_sb")
            wo_sb = pp.tile([P, NCT, C], bf, name="wo_sb")
            bq_sb = pp.tile([P, NCT], f32, name="bq_sb")
            ev_sb = pp.tile([9, H, 65], bf, name="ev_sb")
            o2_sb = pp.tile([P, P], fp16, name="o2_sb")
            cor1 = pp.tile([P, H, 5], bf, name="cor1")
            cor2 = pp.tile([32, H, 3], bf, name="cor2")
            mk_sb = pp.tile([9, TQ], bf, name="mk_sb")

            q_sb = pp.tile([P, NCT, TQ], bf, name="q_sb")
            k_sb = pp.tile([P, NCT, T], bf, name="k_sb")
            vT65 = pp.tile([P, 8, H, 65], bf, name="vT65")  # [j, tb, h, ch|1]
            ctxn = pp.tile([P, NCT, TQ], bf, name="ctxn")

            # ---- input loads, in first-use order -----------------------
            nc.sync.dma_start(xq_sb[:], xq[:])
            nc.sync.dma_start(wq_sb[:], wqT[:])
            nc.sync.dma_start(bq_sb[:], bq8[:])
            nc.sync.dma_start(xk_sb[:], xk[:])
            nc.sync.dma_start(wk_sb[:], wkT[:])
            nc.sync.dma_start(xv_sb[:], xv[:])
            nc.sync.dma_start(wv_sb[:], wvT[:])
            nc.scalar.dma_start(ev_sb[:], evh[:])
            nc.scalar.dma_start(o2_sb[64:66, :], oneh2[:])
            nc.scalar.dma_start(cor1[96:128, :, :], cor1d[:])
            nc.scalar.dma_start(cor2[:], cor2d[:])
            nc.scalar.dma_start(mk_sb[:], maskd[:])
            nc.sync.dma_start(wo_sb[:], woT[:])

            # ones column of the v stationary (Z accumulator).  v and ev are
            # host-scaled by 1/64 and the ones column holds 1/64 so that
            # 1/Z' = 64/Z stays in fp16 normal range for the broadcast matmul;
            # the factors cancel exactly in ctxn = (ctx/64) * (64/Z).
            nc.vector.memset(vT65[:, :, :, 64:65], 1.0 / 64.0)

            # ---- helpers ----------------------------------------------
            def projqk(ct):
                pq = psP.tile([P, TQ], f32, name="pj")
                for ci in range(NCT):
                    nc.tensor.matmul(pq, wq_sb[:, ci, ct * P:(ct + 1) * P],
                                     xq_sb[:, ci, :], start=(ci == 0), stop=(ci == 3))
                nc.vector.tensor_scalar_add(q_sb[:, ct, :], pq, bq_sb[:, ct:ct + 1])
                for ns in range(2):
                    pk = psP.tile([P, TQ], f32, name="pj")
                    for ci in range(NCT):
                        nc.tensor.matmul(pk, wk_sb[:, ci, ct * P:(ct + 1) * P],
                                         xk_sb[:, ci, ns * TQ:(ns + 1) * TQ],
                                         start=(ci == 0), stop=(ci == 3))
                    nc.vector.tensor_copy(k_sb[:, ct, ns * TQ:(ns + 1) * TQ], pk)

            def projv(tb):
                pv = psP.tile([P, TQ], f32, name="pj")
                for ci in range(NCT):
                    nc.tensor.matmul(pv, xv_sb[:, ci, tb * P:(tb + 1) * P],
                                     wv_sb[:, ci, :], start=(ci == 0), stop=(ci == 3))
                # pv: [128 t, 512 ch] -> vT65[:, tb, h, 0:64]
                nc.vector.tensor_copy(
                    vT65[:, tb, :, 0:64],
                    pv[:].rearrange("p (h c) -> p h c", h=H))

            e_tiles = {}

            def score_unit(a, jb):
                ps = psS.tile([P, 2 * TQ], f32, name="sc")
                for hh in range(2):
                    nc.tensor.matmul(ps[:, hh * TQ:(hh + 1) * TQ],
                                     k_sb[hh * D:(hh + 1) * D, a, jb * P:(jb + 1) * P],
                                     q_sb[hh * D:(hh + 1) * D, a, :],
                                     start=True, stop=True)
                if jb == 7:
                    for hh in range(2):
                        nc.vector.tensor_tensor(
                            out=ps[96:128, hh * TQ:hh * TQ + 5],
                            in0=ps[96:128, hh * TQ:hh * TQ + 5],
                            in1=cor1[96:128, 2 * a + hh, :], op=add_op)
                if jb == 4:
                    for hh in range(2):
                        nc.vector.tensor_tensor(
                            out=ps[0:32, hh * TQ + 509:hh * TQ + 512],
                            in0=ps[0:32, hh * TQ + 509:hh * TQ + 512],
                            in1=cor2[0:32, 2 * a + hh, :], op=add_op)
                e_a = e_tiles[a]
                nc.scalar.activation(
                    e_a[:, :, jb, :],
                    ps[:].rearrange("p (u q) -> p u q", u=2), EXP)

            def stage_band(a):
                e_a = e_tiles[a]
                abTa = wp.tile([9, IB * P], bf, name="abTa")
                abTb = wp.tile([9, IB * P], bf, name="abTb")
                for hh in range(2):
                    # rows 4..1027 <- e[j, i]; row = j + 4
                    nc.gpsimd.dma_start(
                        stg[a, 4:1028, hh, :].rearrange("(jb p) i -> p jb i", p=P),
                        e_a[:, hh, :, :])
                    # wrap halo rows 0..3 <- j 1020..1023
                    nc.gpsimd.dma_start(stg[a, 0:4, hh, :],
                                        e_a[124:128, hh, 7, :])
                # diagonal readback: abT*[m, i] = stg[a, i+m, hh, i].
                # The two gathers are element-scattered (slow transfers), so
                # run them concurrently on different queues into separate tiles.
                diag0 = AP(tensor=stg[:].tensor,
                           offset=a * (1028 * 2 * TQ),
                           ap=[[2 * TQ, 9], [2 * TQ + 1, TQ]])
                nc.sync.dma_start(abTa[:], diag0)
                diag1 = AP(tensor=stg[:].tensor,
                           offset=a * (1028 * 2 * TQ) + TQ,
                           ap=[[2 * TQ, 9], [2 * TQ + 1, TQ]])
                nc.gpsimd.dma_start(abTb[:], diag1)
                nc.vector.tensor_tensor(out=abTa[:], in0=abTa[:], in1=mk_sb[:],
                                        op=mul_op)
                nc.vector.tensor_tensor(out=abTb[:], in0=abTb[:], in1=mk_sb[:],
                                        op=mul_op)
                return abTa, abTb

            def ctx_head(a, hh):
                e_a = e_tiles[a]
                h = 2 * a + hh
                pc = psC.tile([P, TQ], f32, name="pc")
                for jb in range(8):
                    nc.tensor.matmul(pc[0:65, :], vT65[:, jb, h, :],
                                     e_a[:, hh, jb, :],
                                     start=(jb == 0), stop=False,
                                     skip_group_check=True)
                return pc

            def band_head(a, hh, ab, pc):
                nc.tensor.matmul(pc[0:65, :], ev_sb[:, 2 * a + hh, :], ab[:],
                                 start=False, stop=True, skip_group_check=True)

            def finish_pair(a, pc0, pc1):
                z2 = wp.tile([P, TQ], f32, name="z2")
                zt = wp.tile([P, TQ], f32, name="zt")
                r2 = wp.tile([P, TQ], f32, name="r2")
                r16 = wp.tile([P, TQ], fp16, name="r16")
                # assemble both Z rows at partitions 64..65, one reciprocal
                nc.scalar.copy(z2[64:65, :], pc0[64:65, :])
                nc.scalar.copy(zt[64:65, :], pc1[64:65, :])
                nc.sync.dma_start(z2[65:66, :], zt[64:65, :])
                nc.vector.reciprocal(r2[64:66, :], z2[64:66, :])
                nc.vector.tensor_copy(r16[64:66, :], r2[64:66, :])
                pz = psP.tile([P, TQ], f32, name="pj")
                nc.tensor.matmul(pz, o2_sb[64:66, :], r16[64:66, :],
                                 start=True, stop=True)
                rzb = wp.tile([P, TQ], f32, name="rzb")
                nc.vector.tensor_copy(rzb[:], pz[:])
                # head 0: aligned multiply into ctxn rows 0..63
                nc.vector.tensor_tensor(out=ctxn[0:D, a, :], in0=pc0[0:D, :],
                                        in1=rzb[0:D, :], op=mul_op)
                # head 1: multiply at partitions 0..63, then DMA-shift to 64..127
                ctmp = wp.tile([D, TQ], bf, name="ctmp")
                nc.vector.tensor_tensor(out=ctmp[:], in0=pc1[0:D, :],
                                        in1=rzb[D:2 * D, :], op=mul_op)
                nc.sync.dma_start(ctxn[D:2 * D, a, :], ctmp[:])

            def oproj(co):
                po = psP.tile([P, TQ], f32, name="pj")
                for ci in range(NCT):
                    nc.tensor.matmul(po, wo_sb[:, ci, co * P:(co + 1) * P],
                                     ctxn[:, ci, :], start=(ci == 0), stop=(ci == 3))
                o_sb = wp.tile([P, TQ], f32, name="o_sb")
                nc.vector.tensor_copy(o_sb[:], po)
                nc.sync.dma_start(Out[co], o_sb[:])

            # ---- emission: software-pipelined --------------------------
            for a in range(NCT):
                e_tiles[a] = ep.tile([P, 2, 8, TQ], bf, name="e_a")

            projqk(0)
            score_unit(0, 0); score_unit(0, 1)
            projqk(1)
            score_unit(0, 2); score_unit(0, 3)
            projv(0); projv(1); projv(2); projv(3)
            score_unit(0, 4); score_unit(0, 5)
            projv(4); projv(5); projv(6); projv(7)
            score_unit(0, 6); score_unit(0, 7)

            projqk(2)
            for jb in range(4):
                score_unit(1, jb)
            abT0a, abT0b = stage_band(0)
            pc00 = ctx_head(0, 0)
            score_unit(1, 4); score_unit(1, 5)
            pc01 = ctx_head(0, 1)
            band_head(0, 0, abT0a, pc00); band_head(0, 1, abT0b, pc01)
            score_unit(1, 6); score_unit(1, 7)
            finish_pair(0, pc00, pc01)

            projqk(3)
            for jb in range(4):
                score_unit(2, jb)
            abT1a, abT1b = stage_band(1)
            pc10 = ctx_head(1, 0)
            score_unit(2, 4); score_unit(2, 5)
            pc11 = ctx_head(1, 1)
            band_head(1, 0, abT1a, pc10); band_head(1, 1, abT1b, pc11)
            score_unit(2, 6); score_unit(2, 7)
            finish_pair(1, pc10, pc11)

            for jb in range(4):
                score_unit(3, jb)
            abT2a, abT2b = stage_band(2)
            pc20 = ctx_head(2, 0)
            score_unit(3, 4); score_unit(3, 5)
            pc21 = ctx_head(2, 1)
            band_head(2, 0, abT2a, pc20); band_head(2, 1, abT2b, pc21)
            score_unit(3, 6); score_unit(3, 7)
            finish_pair(2, pc20, pc21)

            abT3a, abT3b = stage_band(3)
            pc30 = ctx_head(3, 0)
            pc31 = ctx_head(3, 1)
            band_head(3, 0, abT3a, pc30); band_head(3, 1, abT3b, pc31)
            finish_pair(3, pc30, pc31)

            for co in range(NCT):
                oproj(co)

    nc.compile()
    return nc


def _prep(inputs):
    x_q = np.asarray(inputs["x_q"], np.float32)
    x_k = np.asarray(inputs["x_k"], np.float32)
    x_v = np.asarray(inputs["x_v"], np.float32)
    Wq = np.asarray(inputs["Wq"], np.float32)
    Wk = np.asarray(inputs["Wk"], np.float32)
    Wv = np.asarray(inputs["Wv"], np.float32)
    Wo = np.asarray(inputs["Wo"], np.float32)
    bq = np.asarray(inputs["bq"], np.float32)
    ek = np.asarray(inputs["emb_rel_k"], np.float32)
    ev = np.asarray(inputs["emb_rel_v"], np.float32)

    def ctile(a):  # (C, X) -> (P, NCT, X) partition-first
        return np.ascontiguousarray(a.reshape(NCT, P, -1).transpose(1, 0, 2))

    wqT = ctile(Wq.T * 0.125).astype(BF16)
    wkT = ctile(Wk.T).astype(BF16)
    wvT = ctile(Wv.T * (1.0 / 64.0)).astype(BF16)
    woT = ctile(Wo.T).astype(BF16)
    bq8 = np.ascontiguousarray((bq * 0.125).reshape(NCT, P).T).astype(np.float32)

    # band stationary per head: rows 9*(h%2)+m carry ev[h, m, :], col 64 = 0
    evh = np.zeros((9, H, 65), np.float32)
    for h in range(H):
        for m in range(9):
            evh[m, h, 0:D] = ev[h, m] * (1.0 / 64.0)
    evh = evh.astype(BF16)

    # broadcast stationary (lives at sbuf partitions 64..65)
    oneh2 = np.zeros((2, P), np.float16)
    oneh2[0, 0:D] = 1.0
    oneh2[1, D:2 * D] = 1.0

    in_maps = []
    for core in range(8):
        b, half = core // 2, core % 2
        i0 = half * TQ
        # corner score terms, computed on host from a few q columns
        cor1 = np.zeros((P, H, 5), np.float32)
        cor2 = np.zeros((P, H, 3), np.float32)
        if half == 0:
            qc = (Wq @ x_q[b][:, 0:5]) * 0.125 + (bq[:, None] * 0.125)
            for h in range(H):
                for p in range(5):
                    for c in range(p, 5):
                        # j = 1019 + c, i = p, emb row m = c - p
                        cor1[p, h, c] = qc[h * D:(h + 1) * D, p] @ ek[h, c - p]
        else:
            qc = (Wq @ x_q[b][:, 1019:1023]) * 0.125 + (bq[:, None] * 0.125)
            for h in range(H):
                for p in (125, 126, 127):
                    for c in range(3):
                        m = 133 + c - p
                        if 6 <= m <= 8:
                            # i_glob = 896 + p, uses q_{i-1} = col 895 + p
                            cor2[p, h, c] = qc[h * D:(h + 1) * D, (895 + p) - 1019] @ ek[h, m]
        # j-major transposed corner tables, zero-padded to 32 aligned rows
        cor1T = np.zeros((32, H, 5), np.float32)
        cor1T[27:32] = cor1[0:5].transpose(2, 1, 0)     # rows 123..127 <- [cj, h, i]
        cor2T = np.zeros((32, H, 3), np.float32)
        cor2T[0:3] = cor2[125:128].transpose(2, 1, 0)   # rows 0..2    <- [cj, h, i]

        # band validity mask: row 9*hh+m, col i_abs
        maskT = np.zeros((9, TQ), np.float32)
        ivec = np.arange(TQ)
        for m in range(9):
            maskT[m, :] = ((i0 + ivec + m - 4 >= 0) & (i0 + ivec + m - 4 < T))

        in_maps.append({
            "xq": ctile(x_q[b][:, i0:i0 + TQ]).astype(BF16),
            "xk": ctile(np.roll(x_k[b], -i0, axis=1)).astype(BF16),
            "xv": ctile(np.roll(x_v[b], -i0, axis=1)).astype(BF16),
            "wqT": wqT, "wkT": wkT, "wvT": wvT, "woT": woT,
            "bq8": bq8, "evh": evh, "oneh2": oneh2,
            "cor1d": cor1T.astype(BF16), "cor2d": cor2T.astype(BF16),
            "maskd": maskT.astype(BF16),
        })
    return in_maps


def kernel(**inputs):
    if "nc" not in _CACHE:
        _CACHE["nc"] = build_nc()
    nc = _CACHE["nc"]
    in_maps = _prep(inputs)
    res = run_bass_kernel_spmd(nc, in_maps, list(range(8)))
    bo = np.asarray(inputs["bo"], np.float32)
    bv = np.asarray(inputs["bv"], np.float32)
    Wo = np.asarray(inputs["Wo"], np.float32)
    bo_eff = bo + Wo @ bv
    out = np.zeros((4, C, T), np.float32)
    for core in range(8):
        b, half = core // 2, core % 2
        o = np.asarray(res.results[core]["Out"]).reshape(C, TQ)
        out[b][:, half * TQ:(half + 1) * TQ] = o
    out += bo_eff[None, :, None]
    return out


# revision 30
# speedup vs baseline: 2.1313x; 1.9296x over previous
"""Trainium2 Bass kernel for nn_MultiHeadAttention_37477884625313.

Multi-head attention (B=4, C=512, T=1024, H=8, d=64) with the reference's
relative-position terms:
  - score-side rel term lands at 21 corner entries per head
  - ctx-side rel term is the +/-4 diagonal band of attn with emb_v

Sharding: 8 cores = 4 batches x 2 query-halves.  k/v time axes are
host-rotated by -i0 so the SPMD program is identical across cores.

v2: scores are computed j-major (keys on partitions) so no transposes are
needed before the ctx matmul; the softmax denominator Z comes from a ones
column appended to the v stationary; exp is the only scalar-engine work;
the rel_v band is gathered via a DRAM diagonal read on the gpsimd queue.
"""

import sys

sys.path.insert(0, "/opt/trn_rl_repo")

import numpy as np
import ml_dtypes

import concourse.bass as bass
import concourse.mybir as mybir
import concourse.tile as tile
from concourse import bacc
from concourse.ap import AP
from concourse.masks import make_identity
from concourse.bass_utils import run_bass_kernel_spmd

BF16 = ml_dtypes.bfloat16
P = 128
C = 512
T = 1024
H = 8
D = 64
TQ = 512          # queries per core
IB = 4            # query i-blocks per core
NCT = 4           # channel tiles (512 / 128)
WJ = 136          # band j-window rows per i-block (4 + 128 + 4)
EXP = mybir.ActivationFunctionType.Exp

_CACHE = {}


def build_nc():
    nc = bacc.Bacc("TRN2", target_bir_lowering=False)
    f32, bf = mybir.dt.float32, mybir.dt.bfloat16
    add_op = mybir.AluOpType.add
    mul_op = mybir.AluOpType.mult

    xq = nc.declare_dram_parameter("xq", [P, NCT, TQ], bf, isOutput=False)
    xk = nc.declare_dram_parameter("xk", [P, NCT, T], bf, isOutput=False)
    xv = nc.declare_dram_parameter("xv", [P, NCT, T], bf, isOutput=False)
    wqT = nc.declare_dram_parameter("wqT", [P, NCT, C], bf, isOutput=False)
    wkT = nc.declare_dram_parameter("wkT", [P, NCT, C], bf, isOutput=False)
    wvT = nc.declare_dram_parameter("wvT", [P, NCT, C], bf, isOutput=False)
    woT = nc.declare_dram_parameter("woT", [P, NCT, C], bf, isOutput=False)
    bq8 = nc.declare_dram_parameter("bq8", [P, NCT], f32, isOutput=False)
    evh = nc.declare_dram_parameter("evh", [9, H, 65], bf, isOutput=False)
    fp16 = mybir.dt.float16
    oneh2 = nc.declare_dram_parameter("oneh2", [2, P], fp16, isOutput=False)
    cor1d = nc.declare_dram_parameter("cor1d", [32, H, 5], bf, isOutput=False)
    cor2d = nc.declare_dram_parameter("cor2d", [32, H, 3], bf, isOutput=False)
    maskd = nc.declare_dram_parameter("maskd", [P, 36], bf, isOutput=False)
    Out = nc.declare_dram_parameter("Out", [NCT, P, TQ], f32, isOutput=True)

    # band staging: per pair a: i-major band-score exp windows [i, hh, ib*136+jw]
    Lb = nc.dram_tensor("Lb", [NCT, P, 2, 544], bf)

    with tile.TileContext(nc) as tc:
        with (
            tc.tile_pool(name="persist", bufs=1) as pp,
            tc.tile_pool(name="epool", bufs=2) as ep,
            tc.tile_pool(name="work", bufs=2) as wp,
            tc.tile_pool(name="psP", bufs=1, space="PSUM") as psP,
            tc.tile_pool(name="psS", bufs=2, space="PSUM") as psS,
            tc.tile_pool(name="psC", bufs=2, space="PSUM") as psC,
            tc.tile_pool(name="psB", bufs=1, space="PSUM") as psB,
        ):
            # ---- persistent tiles --------------------------------------
            xq_sb = pp.tile([P, NCT, TQ], bf, name="xq_sb")
            xk_sb = pp.tile([P, NCT, T], bf, name="xk_sb")
            xv_sb = pp.tile([P, NCT, T], bf, name="xv_sb")
            wq_sb = pp.tile([P, NCT, C], bf, name="wq_sb")
            wk_sb = pp.tile([P, NCT, C], bf, name="wk_sb")
            wv_sb = pp.tile([P, NCT, C], bf, name="wv_sb")
            wo_sb = pp.tile([P, NCT, C], bf, name="wo_sb")
            bq_sb = pp.tile([P, NCT], f32, name="bq_sb")
            ev_sb = pp.tile([9, H, 65], bf, name="ev_sb")
            o2_sb = pp.tile([P, P], fp16, name="o2_sb")
            cor1 = pp.tile([P, H, 5], bf, name="cor1")
            cor2 = pp.tile([32, H, 3], bf, name="cor2")
            mk_sb = pp.tile([P, 36], bf, name="mk_sb")

            q_sb = pp.tile([P, NCT, TQ], bf, name="q_sb")
            k_sb = pp.tile([P, NCT, T], bf, name="k_sb")
            vT65 = pp.tile([P, 8, H, 65], bf, name="vT65")  # [j, tb, h, ch|1]
            ctxn = pp.tile([P, NCT, TQ], bf, name="ctxn")

            # ---- input loads, in first-use order -----------------------
            nc.sync.dma_start(xq_sb[:], xq[:])
            nc.sync.dma_start(wq_sb[:], wqT[:])
            nc.sync.dma_start(bq_sb[:], bq8[:])
            nc.sync.dma_start(xk_sb[:], xk[:])
            nc.sync.dma_start(wk_sb[:], wkT[:])
            nc.sync.dma_start(xv_sb[:], xv[:])
            nc.sync.dma_start(wv_sb[:], wvT[:])
            nc.scalar.dma_start(ev_sb[:], evh[:])
            nc.scalar.dma_start(o2_sb[64:66, :], oneh2[:])
            nc.scalar.dma_start(cor1[96:128, :, :], cor1d[:])
            nc.scalar.dma_start(cor2[:], cor2d[:])
            nc.scalar.dma_start(mk_sb[:], maskd[:])
            nc.sync.dma_start(wo_sb[:], woT[:])

            # ones column of the v stationary (Z accumulator).  v and ev are
            # host-scaled by 1/64 and the ones column holds 1/64 so that
            # 1/Z' = 64/Z stays in fp16 normal range for the broadcast matmul;
            # the factors cancel exactly in ctxn = (ctx/64) * (64/Z).
            nc.vector.memset(vT65[:, :, :, 64:65], 1.0 / 64.0)
            idbf = pp.tile([P, P], bf, name="idbf")
            make_identity(nc, idbf[:])

            # ---- helpers ----------------------------------------------
            def projqk(ct):
                pq = psP.tile([P, TQ], f32, name="pj")
                for ci in range(NCT):
                    nc.tensor.matmul(pq, wq_sb[:, ci, ct * P:(ct + 1) * P],
                                     xq_sb[:, ci, :], start=(ci == 0), stop=(ci == 3))
                nc.vector.tensor_scalar_add(q_sb[:, ct, :], pq, bq_sb[:, ct:ct + 1])
                for ns in range(2):
                    pk = psP.tile([P, TQ], f32, name="pj")
                    for ci in range(NCT):
                        nc.tensor.matmul(pk, wk_sb[:, ci, ct * P:(ct + 1) * P],
                                         xk_sb[:, ci, ns * TQ:(ns + 1) * TQ],
                                         start=(ci == 0), stop=(ci == 3))
                    nc.vector.tensor_copy(k_sb[:, ct, ns * TQ:(ns + 1) * TQ], pk)

            def projv(tb):
                pv = psP.tile([P, TQ], f32, name="pj")
                for ci in range(NCT):
                    nc.tensor.matmul(pv, xv_sb[:, ci, tb * P:(tb + 1) * P],
                                     wv_sb[:, ci, :], start=(ci == 0), stop=(ci == 3))
                # pv: [128 t, 512 ch] -> vT65[:, tb, h, 0:64]
                nc.vector.tensor_copy(
                    vT65[:, tb, :, 0:64],
                    pv[:].rearrange("p (h c) -> p h c", h=H))

            e_tiles = {}

            def score_unit(a, jb):
                ps = psS.tile([P, 2 * TQ], f32, name="sc")
                for hh in range(2):
                    nc.tensor.matmul(ps[:, hh * TQ:(hh + 1) * TQ],
                                     k_sb[hh * D:(hh + 1) * D, a, jb * P:(jb + 1) * P],
                                     q_sb[hh * D:(hh + 1) * D, a, :],
                                     start=True, stop=True)
                if jb == 7:
                    for hh in range(2):
                        nc.vector.tensor_tensor(
                            out=ps[96:128, hh * TQ:hh * TQ + 5],
                            in0=ps[96:128, hh * TQ:hh * TQ + 5],
                            in1=cor1[96:128, 2 * a + hh, :], op=add_op)
                if jb == 4:
                    for hh in range(2):
                        nc.vector.tensor_tensor(
                            out=ps[0:32, hh * TQ + 509:hh * TQ + 512],
                            in0=ps[0:32, hh * TQ + 509:hh * TQ + 512],
                            in1=cor2[0:32, 2 * a + hh, :], op=add_op)
                e_a = e_tiles[a]
                nc.scalar.activation(
                    e_a[:, :, jb, :],
                    ps[:].rearrange("p (u q) -> p u q", u=2), EXP)

            eb_tiles = {}

            def band_scores(a):
                """Recompute the +/-4 band scores i-major (q stationary), exp
                them, and round-trip through DRAM so the diagonal lands as
                contiguous 9-element runs."""
                for hh in range(2):
                    pb = psS.tile([P, 2 * TQ], f32, name="sc")
                    qst = q_sb[hh * D:(hh + 1) * D, a, :]
                    kst = k_sb[hh * D:(hh + 1) * D, a, :]
                    for ib in range(IB):
                        if ib == 0:
                            nc.tensor.matmul(pb[:, 0:4], qst[:, 0:P],
                                             kst[:, T - 4:T],
                                             start=True, stop=True)
                            nc.tensor.matmul(pb[:, 4:136], qst[:, 0:P],
                                             kst[:, 0:132],
                                             start=True, stop=True)
                        else:
                            nc.tensor.matmul(
                                pb[:, ib * 136:ib * 136 + 136],
                                qst[:, ib * P:(ib + 1) * P],
                                kst[:, ib * P - 4:ib * P + 132],
                                start=True, stop=True)
                    eb = wp.tile([P, 544], bf, name=f"eb{hh}")
                    nc.scalar.activation(eb[:], pb[:, 0:544], EXP)
                    nc.gpsimd.dma_start(Lb[a, :, hh, :], eb[:])
                abI0 = wp.tile([P, 36], bf, name="abI0")
                abI1 = wp.tile([P, 36], bf, name="abI1")
                for hh, abI in ((0, abI0), (1, abI1)):
                    diag = AP(tensor=Lb[:].tensor,
                              offset=a * (P * 2 * 544) + hh * 544,
                              ap=[[2 * 544 + 1, P], [136, IB], [1, 9]])
                    eng = nc.sync if hh == 0 else nc.gpsimd
                    eng.dma_start(
                        abI[:].rearrange("p (b m) -> p b m", b=IB), diag)
                nc.vector.tensor_tensor(out=abI0[:], in0=abI0[:], in1=mk_sb[:],
                                        op=mul_op)
                nc.vector.tensor_tensor(out=abI1[:], in0=abI1[:], in1=mk_sb[:],
                                        op=mul_op)
                eb_tiles[a] = (abI0, abI1)

            def band_transpose(a):
                """abI [128 i, (ib, m)] -> abT [9 m, 512 i] via PE transposes."""
                out = []
                for hh in range(2):
                    abI = eb_tiles[a][hh]
                    pt = psB.tile([16, IB * P], bf, name="pt")
                    for ib in range(IB):
                        nc.tensor.transpose(pt[0:9, ib * P:(ib + 1) * P],
                                            abI[:, ib * 9:(ib + 1) * 9],
                                            idbf[:])
                    abT = wp.tile([16, IB * P], bf, name=f"abT{hh}")
                    nc.vector.tensor_copy(abT[0:9, :], pt[0:9, :])
                    out.append(abT)
                return out

            def ctx_head(a, hh):
                e_a = e_tiles[a]
                h = 2 * a + hh
                pc = psC.tile([P, TQ], f32, name="pc")
                for jb in range(8):
                    nc.tensor.matmul(pc[0:65, :], vT65[:, jb, h, :],
                                     e_a[:, hh, jb, :],
                                     start=(jb == 0), stop=False,
                                     skip_group_check=True)
                return pc

            def band_head(a, hh, ab, pc):
                nc.tensor.matmul(pc[0:65, :], ev_sb[:, 2 * a + hh, :],
                                 ab[0:9, :],
                                 start=False, stop=True, skip_group_check=True)

            def finish_pair(a, pc0, pc1):
                z2 = wp.tile([P, TQ], f32, name="z2")
                zt = wp.tile([P, TQ], f32, name="zt")
                r2 = wp.tile([P, TQ], f32, name="r2")
                r16 = wp.tile([P, TQ], fp16, name="r16")
                # assemble both Z rows at partitions 64..65, one reciprocal
                nc.scalar.copy(z2[64:65, :], pc0[64:65, :])
                nc.scalar.copy(zt[64:65, :], pc1[64:65, :])
                nc.sync.dma_start(z2[65:66, :], zt[64:65, :])
                nc.vector.reciprocal(r2[64:66, :], z2[64:66, :])
                nc.vector.tensor_copy(r16[64:66, :], r2[64:66, :])
                pz = psP.tile([P, TQ], f32, name="pj")
                nc.tensor.matmul(pz, o2_sb[64:66, :], r16[64:66, :],
                                 start=True, stop=True)
                rzb = wp.tile([P, TQ], f32, name="rzb")
                nc.vector.tensor_copy(rzb[:], pz[:])
                # head 0: aligned multiply into ctxn rows 0..63
                nc.vector.tensor_tensor(out=ctxn[0:D, a, :], in0=pc0[0:D, :],
                                        in1=rzb[0:D, :], op=mul_op)
                # head 1: multiply at partitions 0..63, then DMA-shift to 64..127
                ctmp = wp.tile([D, TQ], bf, name="ctmp")
                nc.vector.tensor_tensor(out=ctmp[:], in0=pc1[0:D, :],
                                        in1=rzb[D:2 * D, :], op=mul_op)
                nc.sync.dma_start(ctxn[D:2 * D, a, :], ctmp[:])

            def oproj(co):
                po = psP.tile([P, TQ], f32, name="pj")
                for ci in range(NCT):
                    nc.tensor.matmul(po, wo_sb[:, ci, co * P:(co + 1) * P],
                                     ctxn[:, ci, :], start=(ci == 0), stop=(ci == 3))
                o_sb = wp.tile([P, TQ], f32, name="o_sb")
                nc.vector.tensor_copy(o_sb[:], po)
                nc.sync.dma_start(Out[co], o_sb[:])

            # ---- emission: software-pipelined --------------------------
            for a in range(NCT):
                e_tiles[a] = ep.tile([P, 2, 8, TQ], bf, name="e_a")

            projqk(0)
            band_scores(0)
            score_unit(0, 0); score_unit(0, 1)
            projqk(1)
            score_unit(0, 2); score_unit(0, 3)
            projv(0); projv(1); projv(2); projv(3)
            score_unit(0, 4); score_unit(0, 5)
            ab0 = band_transpose(0)
            projv(4); projv(5); projv(6); projv(7)
            score_unit(0, 6); score_unit(0, 7)

            projqk(2)
            band_scores(1)
            for jb in range(4):
                score_unit(1, jb)
            pc00 = ctx_head(0, 0)
            band_head(0, 0, ab0[0], pc00)
            score_unit(1, 4); score_unit(1, 5)
            ab1 = band_transpose(1)
            pc01 = ctx_head(0, 1)
            band_head(0, 1, ab0[1], pc01)
            score_unit(1, 6); score_unit(1, 7)
            finish_pair(0, pc00, pc01)

            projqk(3)
            band_scores(2)
            for jb in range(4):
                score_unit(2, jb)
            pc10 = ctx_head(1, 0)
            band_head(1, 0, ab1[0], pc10)
            score_unit(2, 4); score_unit(2, 5)
            ab2 = band_transpose(2)
            pc11 = ctx_head(1, 1)
            band_head(1, 1, ab1[1], pc11)
            score_unit(2, 6); score_unit(2, 7)
            finish_pair(1, pc10, pc11)

            band_scores(3)
            for jb in range(4):
                score_unit(3, jb)
            pc20 = ctx_head(2, 0)
            band_head(2, 0, ab2[0], pc20)
            score_unit(3, 4); score_unit(3, 5)
            ab3 = band_transpose(3)
            pc21 = ctx_head(2, 1)
            band_head(2, 1, ab2[1], pc21)
            score_unit(3, 6); score_unit(3, 7)
            finish_pair(2, pc20, pc21)

            pc30 = ctx_head(3, 0)
            band_head(3, 0, ab3[0], pc30)
            pc31 = ctx_head(3, 1)
            band_head(3, 1, ab3[1], pc31)
            finish_pair(3, pc30, pc31)

            for co in range(NCT):
                oproj(co)

    nc.compile()
    return nc


def _prep(inputs):
    x_q = np.asarray(inputs["x_q"], np.float32)
    x_k = np.asarray(inputs["x_k"], np.float32)
    x_v = np.asarray(inputs["x_v"], np.float32)
    Wq = np.asarray(inputs["Wq"], np.float32)
    Wk = np.asarray(inputs["Wk"], np.float32)
    Wv = np.asarray(inputs["Wv"], np.float32)
    Wo = np.asarray(inputs["Wo"], np.float32)
    bq = np.asarray(inputs["bq"], np.float32)
    ek = np.asarray(inputs["emb_rel_k"], np.float32)
    ev = np.asarray(inputs["emb_rel_v"], np.float32)

    def ctile(a):  # (C, X) -> (P, NCT, X) partition-first
        return np.ascontiguousarray(a.reshape(NCT, P, -1).transpose(1, 0, 2))

    wqT = ctile(Wq.T * 0.125).astype(BF16)
    wkT = ctile(Wk.T).astype(BF16)
    wvT = ctile(Wv.T * (1.0 / 64.0)).astype(BF16)
    woT = ctile(Wo.T).astype(BF16)
    bq8 = np.ascontiguousarray((bq * 0.125).reshape(NCT, P).T).astype(np.float32)

    # band stationary per head: rows 9*(h%2)+m carry ev[h, m, :], col 64 = 0
    evh = np.zeros((9, H, 65), np.float32)
    for h in range(H):
        for m in range(9):
            evh[m, h, 0:D] = ev[h, m] * (1.0 / 64.0)
    evh = evh.astype(BF16)

    # broadcast stationary (lives at sbuf partitions 64..65)
    oneh2 = np.zeros((2, P), np.float16)
    oneh2[0, 0:D] = 1.0
    oneh2[1, D:2 * D] = 1.0

    in_maps = []
    for core in range(8):
        b, half = core // 2, core % 2
        i0 = half * TQ
        # corner score terms, computed on host from a few q columns
        cor1 = np.zeros((P, H, 5), np.float32)
        cor2 = np.zeros((P, H, 3), np.float32)
        if half == 0:
            qc = (Wq @ x_q[b][:, 0:5]) * 0.125 + (bq[:, None] * 0.125)
            for h in range(H):
                for p in range(5):
                    for c in range(p, 5):
                        # j = 1019 + c, i = p, emb row m = c - p
                        cor1[p, h, c] = qc[h * D:(h + 1) * D, p] @ ek[h, c - p]
        else:
            qc = (Wq @ x_q[b][:, 1019:1023]) * 0.125 + (bq[:, None] * 0.125)
            for h in range(H):
                for p in (125, 126, 127):
                    for c in range(3):
                        m = 133 + c - p
                        if 6 <= m <= 8:
                            # i_glob = 896 + p, uses q_{i-1} = col 895 + p
                            cor2[p, h, c] = qc[h * D:(h + 1) * D, (895 + p) - 1019] @ ek[h, m]
        # j-major transposed corner tables, zero-padded to 32 aligned rows
        cor1T = np.zeros((32, H, 5), np.float32)
        cor1T[27:32] = cor1[0:5].transpose(2, 1, 0)     # rows 123..127 <- [cj, h, i]
        cor2T = np.zeros((32, H, 3), np.float32)
        cor2T[0:3] = cor2[125:128].transpose(2, 1, 0)   # rows 0..2    <- [cj, h, i]

        # band validity mask: row 9*hh+m, col i_abs
        maskI = np.zeros((P, 36), np.float32)
        for ib in range(IB):
            for m in range(9):
                ig = i0 + ib * P + np.arange(P)
                maskI[:, ib * 9 + m] = ((ig + m - 4 >= 0) & (ig + m - 4 < T))

        in_maps.append({
            "xq": ctile(x_q[b][:, i0:i0 + TQ]).astype(BF16),
            "xk": ctile(np.roll(x_k[b], -i0, axis=1)).astype(BF16),
            "xv": ctile(np.roll(x_v[b], -i0, axis=1)).astype(BF16),
            "wqT": wqT, "wkT": wkT, "wvT": wvT, "woT": woT,
            "bq8": bq8, "evh": evh, "oneh2": oneh2,
            "cor1d": cor1T.astype(BF16), "cor2d": cor2T.astype(BF16),
            "maskd": maskI.astype(BF16),
        })
    return in_maps


def kernel(**inputs):
    if "nc" not in _CACHE:
        _CACHE["nc"] = build_nc()
    nc = _CACHE["nc"]
    in_maps = _prep(inputs)
    res = run_bass_kernel_spmd(nc, in_maps, list(range(8)))
    bo = np.asarray(inputs["bo"], np.float32)
    bv = np.asarray(inputs["bv"], np.float32)
    Wo = np.asarray(inputs["Wo"], np.float32)
    bo_eff = bo + Wo @ bv
    out = np.zeros((4, C, T), np.float32)
    for core in range(8):
        b, half = core // 2, core % 2
        o = np.asarray(res.results[core]["Out"]).reshape(C, TQ)
        out[b][:, half * TQ:(half + 1) * TQ] = o
    out += bo_eff[None, :, None]
    return out


# revision 31
# speedup vs baseline: 2.4928x; 1.1696x over previous
"""Trainium2 Bass kernel for nn_MultiHeadAttention_37477884625313.

Multi-head attention (B=4, C=512, T=1024, H=8, d=64) with the reference's
relative-position terms:
  - score-side rel term lands at 21 corner entries per head
  - ctx-side rel term is the +/-4 diagonal band of attn with emb_v

Sharding: 8 cores = 4 batches x 2 query-halves.  k/v time axes are
host-rotated by -i0 so the SPMD program is identical across cores.

v2: scores are computed j-major (keys on partitions) so no transposes are
needed before the ctx matmul; the softmax denominator Z comes from a ones
column appended to the v stationary; exp is the only scalar-engine work;
the rel_v band is gathered via a DRAM diagonal read on the gpsimd queue.
"""

import sys

sys.path.insert(0, "/opt/trn_rl_repo")

import numpy as np
import ml_dtypes

import concourse.bass as bass
import concourse.mybir as mybir
import concourse.tile as tile
from concourse import bacc
from concourse.ap import AP
from concourse.masks import make_identity
from concourse.bass_utils import run_bass_kernel_spmd

BF16 = ml_dtypes.bfloat16
P = 128
C = 512
T = 1024
H = 8
D = 64
TQ = 512          # queries per core
IB = 4            # query i-blocks per core
NCT = 4           # channel tiles (512 / 128)
WJ = 136          # band j-window rows per i-block (4 + 128 + 4)
EXP = mybir.ActivationFunctionType.Exp

_CACHE = {}


def build_nc():
    nc = bacc.Bacc("TRN2", target_bir_lowering=False)
    f32, bf = mybir.dt.float32, mybir.dt.bfloat16
    add_op = mybir.AluOpType.add
    mul_op = mybir.AluOpType.mult

    xq = nc.declare_dram_parameter("xq", [P, NCT, TQ], bf, isOutput=False)
    xk = nc.declare_dram_parameter("xk", [P, NCT, T], bf, isOutput=False)
    xv = nc.declare_dram_parameter("xv", [P, NCT, T], bf, isOutput=False)
    wqT = nc.declare_dram_parameter("wqT", [P, NCT, C], bf, isOutput=False)
    wkT = nc.declare_dram_parameter("wkT", [P, NCT, C], bf, isOutput=False)
    wvT = nc.declare_dram_parameter("wvT", [P, NCT, C], bf, isOutput=False)
    woT = nc.declare_dram_parameter("woT", [P, NCT, C], bf, isOutput=False)
    bq8 = nc.declare_dram_parameter("bq8", [P, NCT], f32, isOutput=False)
    evh = nc.declare_dram_parameter("evh", [9, H, 65], bf, isOutput=False)
    fp16 = mybir.dt.float16
    oneh2 = nc.declare_dram_parameter("oneh2", [2, P], fp16, isOutput=False)
    cor1d = nc.declare_dram_parameter("cor1d", [32, H, 5], bf, isOutput=False)
    cor2d = nc.declare_dram_parameter("cor2d", [32, H, 3], bf, isOutput=False)
    maskd = nc.declare_dram_parameter("maskd", [P, 36], bf, isOutput=False)
    Out = nc.declare_dram_parameter("Out", [NCT, P, TQ], f32, isOutput=True)

    # band staging: per pair a: i-major band-score exp windows [i, hh, ib*136+jw]
    Lb = nc.dram_tensor("Lb", [NCT, P, 2, 544], bf)

    with tile.TileContext(nc) as tc:
        with (
            tc.tile_pool(name="persist", bufs=1) as pp,
            tc.tile_pool(name="epool", bufs=2) as ep,
            tc.tile_pool(name="work", bufs=2) as wp,
            tc.tile_pool(name="psP", bufs=1, space="PSUM") as psP,
            tc.tile_pool(name="psS", bufs=2, space="PSUM") as psS,
            tc.tile_pool(name="psC", bufs=2, space="PSUM") as psC,
            tc.tile_pool(name="psB", bufs=1, space="PSUM") as psB,
        ):
            # ---- persistent tiles --------------------------------------
            xq_sb = pp.tile([P, NCT, TQ], bf, name="xq_sb")
            xk_sb = pp.tile([P, NCT, T], bf, name="xk_sb")
            xv_sb = pp.tile([P, NCT, T], bf, name="xv_sb")
            wq_sb = pp.tile([P, NCT, C], bf, name="wq_sb")
            wk_sb = pp.tile([P, NCT, C], bf, name="wk_sb")
            wv_sb = pp.tile([P, NCT, C], bf, name="wv_sb")
            wo_sb = pp.tile([P, NCT, C], bf, name="wo_sb")
            bq_sb = pp.tile([P, NCT], f32, name="bq_sb")
            ev_sb = pp.tile([9, H, 65], bf, name="ev_sb")
            o2_sb = pp.tile([P, P], fp16, name="o2_sb")
            cor1 = pp.tile([P, H, 5], bf, name="cor1")
            cor2 = pp.tile([32, H, 3], bf, name="cor2")
            mk_sb = pp.tile([P, 36], bf, name="mk_sb")

            q_sb = pp.tile([P, NCT, TQ], bf, name="q_sb")
            k_sb = pp.tile([P, NCT, T], bf, name="k_sb")
            vT65 = pp.tile([P, 8, H, 65], bf, name="vT65")  # [j, tb, h, ch|1]
            ctxn = pp.tile([P, NCT, TQ], bf, name="ctxn")

            # ---- input loads, in first-use order -----------------------
            nc.sync.dma_start(xq_sb[:], xq[:])
            nc.sync.dma_start(wq_sb[:], wqT[:])
            nc.sync.dma_start(bq_sb[:], bq8[:])
            nc.sync.dma_start(xk_sb[:], xk[:])
            nc.sync.dma_start(wk_sb[:], wkT[:])
            nc.sync.dma_start(xv_sb[:], xv[:])
            nc.sync.dma_start(wv_sb[:], wvT[:])
            nc.scalar.dma_start(ev_sb[:], evh[:])
            nc.scalar.dma_start(o2_sb[64:66, :], oneh2[:])
            nc.scalar.dma_start(cor1[96:128, :, :], cor1d[:])
            nc.scalar.dma_start(cor2[:], cor2d[:])
            nc.scalar.dma_start(mk_sb[:], maskd[:])
            nc.sync.dma_start(wo_sb[:], woT[:])

            # ones column of the v stationary (Z accumulator).  v and ev are
            # host-scaled by 1/64 and the ones column holds 1/64 so that
            # 1/Z' = 64/Z stays in fp16 normal range for the broadcast matmul;
            # the factors cancel exactly in ctxn = (ctx/64) * (64/Z).
            nc.vector.memset(vT65[:, :, :, 64:65], 1.0 / 64.0)
            idbf = pp.tile([P, P], bf, name="idbf")
            make_identity(nc, idbf[:])

            # ---- helpers ----------------------------------------------
            def projqk(ct):
                pq = psP.tile([P, TQ], f32, name="pj")
                for ci in range(NCT):
                    nc.tensor.matmul(pq, wq_sb[:, ci, ct * P:(ct + 1) * P],
                                     xq_sb[:, ci, :], start=(ci == 0), stop=(ci == 3))
                nc.vector.tensor_scalar_add(q_sb[:, ct, :], pq, bq_sb[:, ct:ct + 1])
                for ns in range(2):
                    pk = psP.tile([P, TQ], f32, name="pj")
                    for ci in range(NCT):
                        nc.tensor.matmul(pk, wk_sb[:, ci, ct * P:(ct + 1) * P],
                                         xk_sb[:, ci, ns * TQ:(ns + 1) * TQ],
                                         start=(ci == 0), stop=(ci == 3))
                    nc.vector.tensor_copy(k_sb[:, ct, ns * TQ:(ns + 1) * TQ], pk)

            def projv(tb):
                pv = psP.tile([P, TQ], f32, name="pj")
                for ci in range(NCT):
                    nc.tensor.matmul(pv, xv_sb[:, ci, tb * P:(tb + 1) * P],
                                     wv_sb[:, ci, :], start=(ci == 0), stop=(ci == 3))
                # pv: [128 t, 512 ch] -> vT65[:, tb, h, 0:64]
                nc.vector.tensor_copy(
                    vT65[:, tb, :, 0:64],
                    pv[:].rearrange("p (h c) -> p h c", h=H))

            e_tiles = {}

            def score_unit(a, jb):
                ps = psS.tile([P, 2 * TQ], f32, name="sc")
                for hh in range(2):
                    nc.tensor.matmul(ps[:, hh * TQ:(hh + 1) * TQ],
                                     k_sb[hh * D:(hh + 1) * D, a, jb * P:(jb + 1) * P],
                                     q_sb[hh * D:(hh + 1) * D, a, :],
                                     start=True, stop=True)
                if jb == 7:
                    for hh in range(2):
                        nc.vector.tensor_tensor(
                            out=ps[96:128, hh * TQ:hh * TQ + 5],
                            in0=ps[96:128, hh * TQ:hh * TQ + 5],
                            in1=cor1[96:128, 2 * a + hh, :], op=add_op)
                if jb == 4:
                    for hh in range(2):
                        nc.vector.tensor_tensor(
                            out=ps[0:32, hh * TQ + 509:hh * TQ + 512],
                            in0=ps[0:32, hh * TQ + 509:hh * TQ + 512],
                            in1=cor2[0:32, 2 * a + hh, :], op=add_op)
                e_a = e_tiles[a]
                nc.scalar.activation(
                    e_a[:, :, jb, :],
                    ps[:].rearrange("p (u q) -> p u q", u=2), EXP)

            eb_tiles = {}

            def band_scores(a):
                """Recompute the +/-4 band scores i-major (q stationary), exp
                them, and round-trip through DRAM so the diagonal lands as
                contiguous 9-element runs."""
                for hh in range(2):
                    pb = psS.tile([P, 2 * TQ], f32, name="sc")
                    qst = q_sb[hh * D:(hh + 1) * D, a, :]
                    kst = k_sb[hh * D:(hh + 1) * D, a, :]
                    for ib in range(IB):
                        if ib == 0:
                            nc.tensor.matmul(pb[:, 0:4], qst[:, 0:P],
                                             kst[:, T - 4:T],
                                             start=True, stop=True)
                            nc.tensor.matmul(pb[:, 4:136], qst[:, 0:P],
                                             kst[:, 0:132],
                                             start=True, stop=True)
                        else:
                            nc.tensor.matmul(
                                pb[:, ib * 136:ib * 136 + 136],
                                qst[:, ib * P:(ib + 1) * P],
                                kst[:, ib * P - 4:ib * P + 132],
                                start=True, stop=True)
                    eb = wp.tile([P, 544], bf, name=f"eb{hh}")
                    nc.scalar.activation(eb[:], pb[:, 0:544], EXP)
                    nc.gpsimd.dma_start(Lb[a, :, hh, :], eb[:])
                abI0 = wp.tile([P, 36], bf, name="abI0")
                abI1 = wp.tile([P, 36], bf, name="abI1")
                for hh, abI in ((0, abI0), (1, abI1)):
                    # raw-AP read of Lb is not dependency-tracked against the
                    # store above, so it MUST stay on the same queue (gpsimd)
                    # to order behind it.
                    diag = AP(tensor=Lb[:].tensor,
                              offset=a * (P * 2 * 544) + hh * 544,
                              ap=[[2 * 544 + 1, P], [136, IB], [1, 9]])
                    nc.gpsimd.dma_start(
                        abI[:].rearrange("p (b m) -> p b m", b=IB), diag)
                nc.vector.tensor_tensor(out=abI0[:], in0=abI0[:], in1=mk_sb[:],
                                        op=mul_op)
                nc.vector.tensor_tensor(out=abI1[:], in0=abI1[:], in1=mk_sb[:],
                                        op=mul_op)
                eb_tiles[a] = (abI0, abI1)

            def band_transpose(a):
                """abI [128 i, (ib, m)] -> abT [9 m, 512 i] via PE transposes."""
                out = []
                for hh in range(2):
                    abI = eb_tiles[a][hh]
                    pt = psB.tile([16, IB * P], bf, name="pt")
                    for ib in range(IB):
                        nc.tensor.transpose(pt[0:9, ib * P:(ib + 1) * P],
                                            abI[:, ib * 9:(ib + 1) * 9],
                                            idbf[:])
                    abT = wp.tile([16, IB * P], bf, name=f"abT{hh}")
                    nc.vector.tensor_copy(abT[0:9, :], pt[0:9, :])
                    out.append(abT)
                return out

            def ctx_head(a, hh):
                e_a = e_tiles[a]
                h = 2 * a + hh
                pc = psC.tile([P, TQ], f32, name="pc")
                for jb in range(8):
                    nc.tensor.matmul(pc[0:65, :], vT65[:, jb, h, :],
                                     e_a[:, hh, jb, :],
                                     start=(jb == 0), stop=False,
                                     skip_group_check=True)
                return pc

            def band_head(a, hh, ab, pc):
                nc.tensor.matmul(pc[0:65, :], ev_sb[:, 2 * a + hh, :],
                                 ab[0:9, :],
                                 start=False, stop=True, skip_group_check=True)

            def finish_pair(a, pc0, pc1):
                z2 = wp.tile([P, TQ], f32, name="z2")
                zt = wp.tile([P, TQ], f32, name="zt")
                r2 = wp.tile([P, TQ], f32, name="r2")
                r16 = wp.tile([P, TQ], fp16, name="r16")
                # assemble both Z rows at partitions 64..65, one reciprocal
                nc.scalar.copy(z2[64:65, :], pc0[64:65, :])
                nc.scalar.copy(zt[64:65, :], pc1[64:65, :])
                nc.sync.dma_start(z2[65:66, :], zt[64:65, :])
                nc.vector.reciprocal(r2[64:66, :], z2[64:66, :])
                nc.vector.tensor_copy(r16[64:66, :], r2[64:66, :])
                pz = psP.tile([P, TQ], f32, name="pj")
                nc.tensor.matmul(pz, o2_sb[64:66, :], r16[64:66, :],
                                 start=True, stop=True)
                rzb = wp.tile([P, TQ], f32, name="rzb")
                nc.vector.tensor_copy(rzb[:], pz[:])
                # head 0: aligned multiply into ctxn rows 0..63
                nc.vector.tensor_tensor(out=ctxn[0:D, a, :], in0=pc0[0:D, :],
                                        in1=rzb[0:D, :], op=mul_op)
                # head 1: multiply at partitions 0..63, then DMA-shift to 64..127
                ctmp = wp.tile([D, TQ], bf, name="ctmp")
                nc.vector.tensor_tensor(out=ctmp[:], in0=pc1[0:D, :],
                                        in1=rzb[D:2 * D, :], op=mul_op)
                nc.sync.dma_start(ctxn[D:2 * D, a, :], ctmp[:])

            def oproj(co):
                po = psP.tile([P, TQ], f32, name="pj")
                for ci in range(NCT):
                    nc.tensor.matmul(po, wo_sb[:, ci, co * P:(co + 1) * P],
                                     ctxn[:, ci, :], start=(ci == 0), stop=(ci == 3))
                o_sb = wp.tile([P, TQ], f32, name="o_sb")
                nc.vector.tensor_copy(o_sb[:], po)
                nc.sync.dma_start(Out[co], o_sb[:])

            # ---- emission: software-pipelined --------------------------
            for a in range(NCT):
                e_tiles[a] = ep.tile([P, 2, 8, TQ], bf, name="e_a")

            projqk(0)
            band_scores(0)
            score_unit(0, 0); score_unit(0, 1)
            projqk(1)
            score_unit(0, 2); score_unit(0, 3)
            projv(0); projv(1); projv(2); projv(3)
            score_unit(0, 4); score_unit(0, 5)
            ab0 = band_transpose(0)
            projv(4); projv(5); projv(6); projv(7)
            score_unit(0, 6); score_unit(0, 7)

            projqk(2)
            band_scores(1)
            for jb in range(4):
                score_unit(1, jb)
            pc00 = ctx_head(0, 0)
            band_head(0, 0, ab0[0], pc00)
            score_unit(1, 4); score_unit(1, 5)
            ab1 = band_transpose(1)
            pc01 = ctx_head(0, 1)
            band_head(0, 1, ab0[1], pc01)
            score_unit(1, 6); score_unit(1, 7)
            finish_pair(0, pc00, pc01)

            projqk(3)
            band_scores(2)
            for jb in range(4):
                score_unit(2, jb)
            pc10 = ctx_head(1, 0)
            band_head(1, 0, ab1[0], pc10)
            score_unit(2, 4); score_unit(2, 5)
            ab2 = band_transpose(2)
            pc11 = ctx_head(1, 1)
            band_head(1, 1, ab1[1], pc11)
            score_unit(2, 6); score_unit(2, 7)
            finish_pair(1, pc10, pc11)

            band_scores(3)
            for jb in range(4):
                score_unit(3, jb)
            pc20 = ctx_head(2, 0)
            band_head(2, 0, ab2[0], pc20)
            score_unit(3, 4); score_unit(3, 5)
            ab3 = band_transpose(3)
            pc21 = ctx_head(2, 1)
            band_head(2, 1, ab2[1], pc21)
            score_unit(3, 6); score_unit(3, 7)
            finish_pair(2, pc20, pc21)

            pc30 = ctx_head(3, 0)
            band_head(3, 0, ab3[0], pc30)
            pc31 = ctx_head(3, 1)
            band_head(3, 1, ab3[1], pc31)
            finish_pair(3, pc30, pc31)

            for co in range(NCT):
                oproj(co)

    nc.compile()
    return nc


def _prep(inputs):
    x_q = np.asarray(inputs["x_q"], np.float32)
    x_k = np.asarray(inputs["x_k"], np.float32)
    x_v = np.asarray(inputs["x_v"], np.float32)
    Wq = np.asarray(inputs["Wq"], np.float32)
    Wk = np.asarray(inputs["Wk"], np.float32)
    Wv = np.asarray(inputs["Wv"], np.float32)
    Wo = np.asarray(inputs["Wo"], np.float32)
    bq = np.asarray(inputs["bq"], np.float32)
    ek = np.asarray(inputs["emb_rel_k"], np.float32)
    ev = np.asarray(inputs["emb_rel_v"], np.float32)

    def ctile(a):  # (C, X) -> (P, NCT, X) partition-first
        return np.ascontiguousarray(a.reshape(NCT, P, -1).transpose(1, 0, 2))

    wqT = ctile(Wq.T * 0.125).astype(BF16)
    wkT = ctile(Wk.T).astype(BF16)
    wvT = ctile(Wv.T * (1.0 / 64.0)).astype(BF16)
    woT = ctile(Wo.T).astype(BF16)
    bq8 = np.ascontiguousarray((bq * 0.125).reshape(NCT, P).T).astype(np.float32)

    # band stationary per head: rows 9*(h%2)+m carry ev[h, m, :], col 64 = 0
    evh = np.zeros((9, H, 65), np.float32)
    for h in range(H):
        for m in range(9):
            evh[m, h, 0:D] = ev[h, m] * (1.0 / 64.0)
    evh = evh.astype(BF16)

    # broadcast stationary (lives at sbuf partitions 64..65)
    oneh2 = np.zeros((2, P), np.float16)
    oneh2[0, 0:D] = 1.0
    oneh2[1, D:2 * D] = 1.0

    in_maps = []
    for core in range(8):
        b, half = core // 2, core % 2
        i0 = half * TQ
        # corner score terms, computed on host from a few q columns
        cor1 = np.zeros((P, H, 5), np.float32)
        cor2 = np.zeros((P, H, 3), np.float32)
        if half == 0:
            qc = (Wq @ x_q[b][:, 0:5]) * 0.125 + (bq[:, None] * 0.125)
            for h in range(H):
                for p in range(5):
                    for c in range(p, 5):
                        # j = 1019 + c, i = p, emb row m = c - p
                        cor1[p, h, c] = qc[h * D:(h + 1) * D, p] @ ek[h, c - p]
        else:
            qc = (Wq @ x_q[b][:, 1019:1023]) * 0.125 + (bq[:, None] * 0.125)
            for h in range(H):
                for p in (125, 126, 127):
                    for c in range(3):
                        m = 133 + c - p
                        if 6 <= m <= 8:
                            # i_glob = 896 + p, uses q_{i-1} = col 895 + p
                            cor2[p, h, c] = qc[h * D:(h + 1) * D, (895 + p) - 1019] @ ek[h, m]
        # j-major transposed corner tables, zero-padded to 32 aligned rows
        cor1T = np.zeros((32, H, 5), np.float32)
        cor1T[27:32] = cor1[0:5].transpose(2, 1, 0)     # rows 123..127 <- [cj, h, i]
        cor2T = np.zeros((32, H, 3), np.float32)
        cor2T[0:3] = cor2[125:128].transpose(2, 1, 0)   # rows 0..2    <- [cj, h, i]

        # band validity mask: row 9*hh+m, col i_abs
        maskI = np.zeros((P, 36), np.float32)
        for ib in range(IB):
            for m in range(9):
                ig = i0 + ib * P + np.arange(P)
                maskI[:, ib * 9 + m] = ((ig + m - 4 >= 0) & (ig + m - 4 < T))

        in_maps.append({
            "xq": ctile(x_q[b][:, i0:i0 + TQ]).astype(BF16),
            "xk": ctile(np.roll(x_k[b], -i0, axis=1)).astype(BF16),
            "xv": ctile(np.roll(x_v[b], -i0, axis=1)).astype(BF16),
            "wqT": wqT, "wkT": wkT, "wvT": wvT, "woT": woT,
            "bq8": bq8, "evh": evh, "oneh2": oneh2,
            "cor1d": cor1T.astype(BF16), "cor2d": cor2T.astype(BF16),
            "maskd": maskI.astype(BF16),
        })
    return in_maps


def kernel(**inputs):
    if "nc" not in _CACHE:
        _CACHE["nc"] = build_nc()
    nc = _CACHE["nc"]
    in_maps = _prep(inputs)
    res = run_bass_kernel_spmd(nc, in_maps, list(range(8)))
    bo = np.asarray(inputs["bo"], np.float32)
    bv = np.asarray(inputs["bv"], np.float32)
    Wo = np.asarray(inputs["Wo"], np.float32)
    bo_eff = bo + Wo @ bv
    out = np.zeros((4, C, T), np.float32)
    for core in range(8):
        b, half = core // 2, core % 2
        o = np.asarray(res.results[core]["Out"]).reshape(C, TQ)
        out[b][:, half * TQ:(half + 1) * TQ] = o
    out += bo_eff[None, :, None]
    return out
